# revision 24
# baseline (speedup 1.0000x reference)
"""Trainium2 Bass kernel for the AtlasMTL model (8-core data parallel).

Model (per sample):
  z1 = relu(x @ W1 + b1); z = relu(z1 @ W2 + b2)
  parent_logits = z @ Wp + bp; child_logits = z @ Wc + bc
  coord_latent = z @ Wlat + blat; coord_umap = z @ Wum + bum
  parent-routed correction on child subset via per-hotspot MLPs.

Key folding: the correction-MLP first layer input is [z | parent_logits |
child_subset], all affine in z, so its pre-activation collapses into one
matmul z @ Wh + bh with Wh/bh assembled on the host. The whole network is
then 4 chained matmuls + relu/bias + an argmax-derived row mask.

Device layout: transposed (features on partitions, batch on the free dim);
host pre-transposes x per shard.  All matmuls run in fp32r (trn2's 20-bit
reduced fp32 at full PE rate); inputs are pre-rounded on the host so the
hardware computes exact products with f32 accumulation.
"""
import os
import sys

sys.path.insert(0, "/opt/trn_rl_repo")

import numpy as np

import concourse.bass as bass
import concourse.tile as tile
from concourse import bacc, mybir
from concourse.bass_utils import run_bass_kernel_spmd

# Problem shapes (hardcoded per harness contract)
B_FULL = 65536
IN = 2000
H1, H2 = 512, 256
P, C = 30, 200
NHOT, CH = 4, 12
FH = 64
N_CORES = 8
BC = B_FULL // N_CORES          # 8192 rows per core
NT = 512                        # batch columns per device tile
NB = BC // NT                   # 16 batch tiles per core
KT1 = (IN + 127) // 128         # 16 k-tiles for stage 1 (last is 80 rows)
HEADS_W = 512                   # [Wp|Wc|Wlat|Wum|pad8|Wh]

F32 = mybir.dt.float32
F32R = mybir.dt.float32r

LAST_RESULTS = None
_NC_CACHE = None


def round_fp32r(a: np.ndarray) -> np.ndarray:
    """Round f32 to fp32r (1s+8e+11m in the top 20 bits), nearest-even."""
    u = np.ascontiguousarray(a, dtype=np.float32).view(np.uint32)
    lsb = (u >> 12) & 1
    u2 = (u + 0x7FF + lsb) & np.uint32(0xFFFFF000)
    return u2.view(np.float32)


def build_nc():
    nc = bacc.Bacc(None)

    # x arrives pre-permuted: [batch-tile b][partition p][k-tile k][col j]
    # so each tile's load is one DMA with a 32 KiB contiguous run per
    # partition (descriptor-efficient).  IN is zero-padded to KT1*128.
    xt_d = nc.dram_tensor("xt", [NB, 128, KT1, NT], F32R, kind="ExternalInput")
    w1_d = nc.dram_tensor("w1", [H1 // 128, 128, KT1, 128], F32R, kind="ExternalInput")
    w2_d = nc.dram_tensor("w2", [H1, H2], F32R, kind="ExternalInput")
    wh_d = nc.dram_tensor("wh", [H2, HEADS_W], F32R, kind="ExternalInput")
    cw2_d = nc.dram_tensor("cw2", [H2, NHOT * CH], F32R, kind="ExternalInput")
    esel_d = nc.dram_tensor("esel", [P, NHOT * CH], F32R, kind="ExternalInput")
    ones_d = nc.dram_tensor("ones48", [1, NHOT * CH], F32R, kind="ExternalInput")
    bias_d = nc.dram_tensor("biases", [128, 11], F32, kind="ExternalInput")

    ident_d = nc.dram_tensor("ident", [128, 128], F32R, kind="ExternalInput")

    pl_o = nc.dram_tensor("plT", [P, BC], F32, kind="ExternalOutput")
    cl_o = nc.dram_tensor("clT", [C, BC], F32, kind="ExternalOutput")
    lat_o = nc.dram_tensor("latT", [16, BC], F32, kind="ExternalOutput")
    um_o = nc.dram_tensor("umT", [2, BC], F32, kind="ExternalOutput")
    z_o = nc.dram_tensor("zT", [H2, BC], F32, kind="ExternalOutput")
    rm_o = nc.dram_tensor("resmT", [NHOT * CH, BC], F32, kind="ExternalOutput")
    mk_o = nc.dram_tensor("maskB", [NB, NT // 128, 128], F32, kind="ExternalOutput")

    AF = mybir.ActivationFunctionType
    AX = mybir.AxisListType
    OP = mybir.AluOpType

    with tile.TileContext(nc) as tc:
        with tc.tile_pool(name="wpool", bufs=1) as wpool, \
             tc.tile_pool(name="xpool", bufs=2) as xpool, \
             tc.tile_pool(name="apool", bufs=2) as apool, \
             tc.tile_pool(name="ps1", bufs=2, space="PSUM") as ps1p, \
             tc.tile_pool(name="ps2", bufs=1, space="PSUM") as ps2p, \
             tc.tile_pool(name="ps3", bufs=2, space="PSUM") as ps3p, \
             tc.tile_pool(name="ps4", bufs=1, space="PSUM") as ps4p, \
             tc.tile_pool(name="psm", bufs=1, space="PSUM") as psmp, \
             tc.tile_pool(name="psmb", bufs=1, space="PSUM") as psmb:

            # ---- stage-1 weights + biases first; the rest are deferred
            # until after tile 0's stage-1 is emitted, so the first matmul
            # only waits on ~5 MB of DMA instead of the full 13 MB.
            bias_sb = wpool.tile([128, 11], F32, tag="biases")
            nc.sync.dma_start(out=bias_sb[:], in_=bias_d[:])
            w1_sb = []
            for m in range(H1 // 128):
                t = wpool.tile([128, KT1, 128], F32R, tag=f"w1_{m}")
                nc.sync.dma_start(out=t[:], in_=w1_d[m])
                w1_sb.append(t)

            w2_sb, wh_sb, cw2_sb = [], [], []
            esel_sb, ones_sb, ident_sb = None, None, None

            def emit_tail_weights():
                nonlocal esel_sb, ones_sb, ident_sb
                for k in range(H1 // 128):
                    t = wpool.tile([128, H2], F32R, tag=f"w2_{k}")
                    nc.sync.dma_start(out=t[:], in_=w2_d[k * 128:(k + 1) * 128])
                    w2_sb.append(t)
                for k in range(H2 // 128):
                    t = wpool.tile([128, HEADS_W], F32R, tag=f"wh_{k}")
                    nc.sync.dma_start(out=t[:], in_=wh_d[k * 128:(k + 1) * 128])
                    wh_sb.append(t)
                for k in range(H2 // 128):
                    t = wpool.tile([128, NHOT * CH], F32R, tag=f"cw2_{k}")
                    nc.sync.dma_start(out=t[:], in_=cw2_d[k * 128:(k + 1) * 128])
                    cw2_sb.append(t)
                esel_sb = wpool.tile([P, NHOT * CH], F32R, tag="esel")
                nc.sync.dma_start(out=esel_sb[:], in_=esel_d[:])
                ones_sb = wpool.tile([1, NHOT * CH], F32R, tag="ones48")
                nc.sync.dma_start(out=ones_sb[:], in_=ones_d[:])
                ident_sb = wpool.tile([128, 128], F32R, tag="ident")
                nc.sync.dma_start(out=ident_sb[:], in_=ident_d[:])

            # ---- batch tiles (software-pipelined emission) ----
            # PE program order per wall-step t:  s1(t) | s2(t-1) | s3+(t-2)
            # so the PE never waits on the ACT eviction latency at stage
            # boundaries — the next tile's stage-1 fills the gap.
            st = {}

            def emit_s1(b):
                xt_sb = xpool.tile([128, KT1, NT], F32R, tag="xt")
                nc.sync.dma_start(out=xt_sb[:], in_=xt_d[b])
                z1_sb = []
                for m in range(H1 // 128):
                    ps = ps1p.tile([128, NT], F32, tag="ps1")
                    for k in range(KT1):
                        nc.tensor.matmul(ps[:], w1_sb[m][:, k, :],
                                         xt_sb[:, k, :], start=(k == 0), stop=(k == KT1 - 1))
                    t = apool.tile([128, NT], F32R, tag=f"z1_{m}")
                    nc.scalar.activation(t[:], ps[:], AF.Relu, bias=bias_sb[:, m:m + 1])
                    z1_sb.append(t)
                st[b] = {"z1": z1_sb}

            def emit_s2(b):
                z1_sb = st[b]["z1"]
                z_sb = []
                for m in range(H2 // 128):
                    ps = ps2p.tile([128, NT], F32, tag="ps2")
                    for k in range(H1 // 128):
                        nc.tensor.matmul(ps[:], w2_sb[k][:, m * 128:(m + 1) * 128],
                                         z1_sb[k][:], start=(k == 0), stop=(k == H1 // 128 - 1))
                    t = apool.tile([128, NT], F32R, tag=f"z_{m}")
                    nc.scalar.activation(t[:], ps[:], AF.Relu, bias=bias_sb[:, 4 + m:5 + m])
                    z_sb.append(t)
                st[b]["z"] = z_sb

            def emit_s3rest(b):
                bs = slice(b * NT, (b + 1) * NT)
                z_sb = st.pop(b)["z"]

                # stage 3: headsT = Wheads.T @ zT + bheads  [512, NT] as 4 tiles
                # rows: 0:30 pl | 30:230 cl | 230:246 lat | 246:248 um | 248:256 pad | 256:512 pre_h
                heads_sb = []
                for m in range(HEADS_W // 128):
                    ps = ps3p.tile([128, NT], F32, tag="ps3")
                    for k in range(H2 // 128):
                        nc.tensor.matmul(ps[:], wh_sb[k][:, m * 128:(m + 1) * 128],
                                         z_sb[k][:], start=(k == 0), stop=(k == H2 // 128 - 1))
                    if m == 0:
                        t = apool.tile([128, NT], F32R, tag="h0")
                        nc.scalar.activation(t[:], ps[:], AF.Identity, bias=bias_sb[:, 6:7])
                    elif m == 1:
                        t = apool.tile([128, NT], F32, tag="h1")
                        nc.scalar.activation(t[:], ps[:], AF.Identity, bias=bias_sb[:, 7:8])
                    else:
                        t = apool.tile([128, NT], F32R, tag=f"h{m}")
                        nc.scalar.activation(t[:], ps[:], AF.Relu, bias=bias_sb[:, 6 + m:7 + m])
                    heads_sb.append(t)

                # stage 4: res48T = cW2blk.T @ hT + cb2   [48, NT]
                ps = ps4p.tile([NHOT * CH, NT], F32, tag="ps4")
                for k in range(H2 // 128):
                    nc.tensor.matmul(ps[:], cw2_sb[k][:], heads_sb[2 + k][:],
                                     start=(k == 0), stop=(k == H2 // 128 - 1))
                res48 = apool.tile([NHOT * CH, NT], F32, tag="res48")
                nc.scalar.activation(res48[:], ps[:], AF.Identity,
                                     bias=bias_sb[:NHOT * CH, 10:11])

                # mask path: argmax over the 30 parent logits, per batch column.
                # pl48 row j = parent_logit[hotspot_ids[j//CH]] (via esel matmul);
                # the max itself is computed in transposed orientation (PE
                # transpose -> DVE free-dim reduce), then broadcast back with
                # K=1 matmuls.  No gpsimd.
                pl = heads_sb[0]
                pl48_ps = psmp.tile([NHOT * CH, NT], F32, tag="psm")
                nc.tensor.matmul(pl48_ps[:], esel_sb[:], pl[:P], start=True, stop=True)
                pl48 = apool.tile([NHOT * CH, NT], F32, tag="pl48")
                nc.scalar.activation(pl48[:], pl48_ps[:], AF.Copy)

                nchunk = NT // 128
                plT_ps = psmb.tile([128, nchunk * P], F32, tag="psb")
                for j in range(nchunk):
                    nc.tensor.transpose(plT_ps[:, j * P:(j + 1) * P],
                                        pl[:P, j * 128:(j + 1) * 128].bitcast(F32),
                                        ident_sb[:P, :P].bitcast(F32))
                plT_v = plT_ps[:].rearrange("p (j c) -> p j c", c=P)
                max30T = apool.tile([128, nchunk], F32, tag="max30T")
                nc.vector.tensor_reduce(max30T[:], plT_v, AX.X, OP.max)
                hot4T = apool.tile([128, nchunk], F32, tag="hot4T")
                nc.vector.tensor_reduce(hot4T[:], plT_v[:, :, 0:NHOT], AX.X, OP.max)
                activeT = apool.tile([128, nchunk], F32, tag="activeT")
                nc.vector.tensor_tensor(activeT[:], hot4T[:], max30T[:], OP.is_ge)

                m30row_ps = psmp.tile([1, NT], F32, tag="psm")
                for j in range(nchunk):
                    nc.tensor.transpose(m30row_ps[:, j * 128:(j + 1) * 128],
                                        max30T[:, j:j + 1], ident_sb[:, :].bitcast(F32))
                m30row_sb = apool.tile([1, NT], F32R, tag="m30")
                nc.scalar.activation(m30row_sb[:], m30row_ps[:], AF.Copy)

                max48_ps = psmp.tile([NHOT * CH, NT], F32, tag="psm")
                nc.tensor.matmul(max48_ps[:], ones_sb[:], m30row_sb[:], start=True, stop=True)

                mask48 = apool.tile([NHOT * CH, NT], F32, tag="mask48")
                nc.vector.tensor_tensor(mask48[:], pl48[:], max48_ps[:], OP.is_equal)
                resm = apool.tile([NHOT * CH, NT], F32, tag="resm")
                nc.vector.tensor_mul(resm[:], mask48[:], res48[:])

                # outputs
                nc.sync.dma_start(out=pl_o[:, bs], in_=heads_sb[0][0:P].bitcast(F32))
                nc.sync.dma_start(out=cl_o[0:98, bs], in_=heads_sb[0][P:128].bitcast(F32))
                nc.sync.dma_start(out=cl_o[98:C, bs], in_=heads_sb[1][0:102])
                nc.sync.dma_start(out=lat_o[:, bs], in_=heads_sb[1][102:118])
                nc.sync.dma_start(out=um_o[:, bs], in_=heads_sb[1][118:120])
                nc.sync.dma_start(out=z_o[0:128, bs], in_=z_sb[0][:].bitcast(F32))
                nc.sync.dma_start(out=z_o[128:H2, bs], in_=z_sb[1][:].bitcast(F32))
                nc.sync.dma_start(out=rm_o[:, bs], in_=resm[:])
                for j in range(NT // 128):
                    nc.sync.dma_start(out=mk_o[b, j, :], in_=activeT[:, j:j + 1])

            for t_ in range(NB + 2):
                if t_ < NB:
                    emit_s1(t_)
                if t_ == 0:
                    emit_tail_weights()
                if 0 <= t_ - 1 < NB:
                    emit_s2(t_ - 1)
                if 0 <= t_ - 2 < NB:
                    emit_s3rest(t_ - 2)

    nc.finalize()
    return nc


def _prep_shared(W1, b1, W2, b2, Wp, bp, Wc, bc, Wlat, blat, Wum, bum,
                 cW1, cb1, cW2, cb2, hotspot_ids, child_idx):
    """Fold correction MLP layer-1 into a single z-matmul; pack weights/biases."""
    f8 = np.float64
    W1, W2, Wp, Wc, Wlat, Wum = [np.asarray(a, np.float32) for a in (W1, W2, Wp, Wc, Wlat, Wum)]
    cW1_, cW2_ = np.asarray(cW1, f8), np.asarray(cW2, f8)
    child = np.asarray(child_idx).reshape(NHOT, CH)
    hots = np.asarray(hotspot_ids).reshape(NHOT)

    Wh = np.zeros((H2, NHOT * FH), f8)
    bh = np.zeros(NHOT * FH, f8)
    cw2blk = np.zeros((NHOT * FH, NHOT * CH), f8)
    cb2flat = np.zeros(NHOT * CH, f8)
    for n in range(NHOT):
        A_n = cW1_[n, :H2, :]
        B_n = cW1_[n, H2:H2 + P, :]
        C_n = cW1_[n, H2 + P:, :]
        Wh[:, n * FH:(n + 1) * FH] = (A_n + np.asarray(Wp, f8) @ B_n
                                      + np.asarray(Wc, f8)[:, child[n]] @ C_n)
        bh[n * FH:(n + 1) * FH] = (np.asarray(bp, f8) @ B_n
                                   + np.asarray(bc, f8)[child[n]] @ C_n
                                   + np.asarray(cb1, f8)[n])
        cw2blk[n * FH:(n + 1) * FH, n * CH:(n + 1) * CH] = cW2_[n]
        cb2flat[n * CH:(n + 1) * CH] = np.asarray(cb2, f8)[n]

    wheads = np.zeros((H2, HEADS_W), np.float32)
    wheads[:, 0:P] = Wp
    wheads[:, P:P + C] = Wc
    wheads[:, 230:246] = Wlat
    wheads[:, 246:248] = Wum
    wheads[:, 256:512] = Wh.astype(np.float32)
    bheads = np.zeros(HEADS_W, np.float32)
    bheads[0:P] = bp
    bheads[P:P + C] = bc
    bheads[230:246] = blat
    bheads[246:248] = bum
    bheads[256:512] = bh.astype(np.float32)

    assert np.array_equal(hots, np.arange(NHOT)), (
        "device mask path hardcodes hotspot_ids == arange(NHOT)")
    esel = np.zeros((P, NHOT * CH), np.float32)
    for j in range(NHOT * CH):
        esel[hots[j // CH], j] = 1.0

    biases = np.zeros((128, 11), np.float32)
    for m in range(4):
        biases[:, m] = np.asarray(b1, np.float32)[m * 128:(m + 1) * 128]
    for m in range(2):
        biases[:, 4 + m] = np.asarray(b2, np.float32)[m * 128:(m + 1) * 128]
    for m in range(4):
        biases[:, 6 + m] = bheads[m * 128:(m + 1) * 128]
    biases[:NHOT * CH, 10] = cb2flat.astype(np.float32)

    w1p = np.zeros((KT1 * 128, H1), np.float32)
    w1p[:IN] = round_fp32r(W1)
    # [k*128+p, m*128+c] -> [m, p, k, c]
    w1p = np.ascontiguousarray(
        w1p.reshape(KT1, 128, H1 // 128, 128).transpose(2, 1, 0, 3))

    return {
        "w1": w1p,
        "w2": round_fp32r(W2),
        "wh": round_fp32r(wheads),
        "cw2": round_fp32r(cw2blk.astype(np.float32)),
        "esel": esel,                      # 0/1: already fp32r-exact
        "ones48": np.ones((1, NHOT * CH), np.float32),
        "biases": biases,
        "ident": np.eye(128, dtype=np.float32),
    }, child


def kernel(x, W1, b1, W2, b2, Wp, bp, Wc, bc, Wlat, blat, Wum, bum,
           cW1, cb1, cW2, cb2, hotspot_ids, child_idx):
    global LAST_RESULTS, _NC_CACHE
    shared, child = _prep_shared(W1, b1, W2, b2, Wp, bp, Wc, bc, Wlat, blat,
                                 Wum, bum, cW1, cb1, cW2, cb2, hotspot_ids, child_idx)

    x = np.asarray(x, np.float32)
    in_maps = []
    for i in range(N_CORES):
        xs = round_fp32r(x[i * BC:(i + 1) * BC])          # [BC, IN]
        xp = np.zeros((BC, KT1 * 128), np.float32)
        xp[:, :IN] = xs
        # [b*NT+j, k*128+p] -> [b, p, k, j]
        xp = np.ascontiguousarray(
            xp.reshape(NB, NT, KT1, 128).transpose(0, 3, 2, 1))
        sh = dict(shared)
        sh["xt"] = xp
        in_maps.append(sh)

    if _NC_CACHE is None:
        _NC_CACHE = build_nc()
    res = run_bass_kernel_spmd(_NC_CACHE, in_maps, list(range(N_CORES)))
    LAST_RESULTS = res

    pl = np.concatenate([np.asarray(r["plT"]).T for r in res.results], axis=0)
    cl = np.concatenate([np.asarray(r["clT"]).T for r in res.results], axis=0)
    lat = np.concatenate([np.asarray(r["latT"]).T for r in res.results], axis=0)
    um = np.concatenate([np.asarray(r["umT"]).T for r in res.results], axis=0)
    z = np.concatenate([np.asarray(r["zT"]).T for r in res.results], axis=0)
    resm = np.concatenate([np.asarray(r["resmT"]).T for r in res.results], axis=0)
    mask = np.concatenate([np.asarray(r["maskB"]).reshape(BC) for r in res.results], axis=0)

    corrected = cl.copy()
    corrected[:, child.ravel()] += resm
    active = mask > 0.5
    return pl, corrected, lat, um, z, active


# revision 25
# speedup vs baseline: 1.0942x; 1.0942x over previous
"""Trainium2 Bass kernel for the AtlasMTL model (8-core data parallel).

Model (per sample):
  z1 = relu(x @ W1 + b1); z = relu(z1 @ W2 + b2)
  parent_logits = z @ Wp + bp; child_logits = z @ Wc + bc
  coord_latent = z @ Wlat + blat; coord_umap = z @ Wum + bum
  parent-routed correction on child subset via per-hotspot MLPs.

Key folding: the correction-MLP first layer input is [z | parent_logits |
child_subset], all affine in z, so its pre-activation collapses into one
matmul z @ Wh + bh with Wh/bh assembled on the host. The whole network is
then 4 chained matmuls + relu/bias + an argmax-derived row mask.

Device layout: transposed (features on partitions, batch on the free dim);
host pre-transposes x per shard.  All matmuls run in fp32r (trn2's 20-bit
reduced fp32 at full PE rate); inputs are pre-rounded on the host so the
hardware computes exact products with f32 accumulation.
"""
import os
import sys

sys.path.insert(0, "/opt/trn_rl_repo")

import numpy as np

import concourse.bass as bass
import concourse.tile as tile
from concourse import bacc, mybir
from concourse.bass_utils import run_bass_kernel_spmd

# Problem shapes (hardcoded per harness contract)
B_FULL = 65536
IN = 2000
H1, H2 = 512, 256
P, C = 30, 200
NHOT, CH = 4, 12
FH = 64
N_CORES = 8
BC = B_FULL // N_CORES          # 8192 rows per core
NT = 512                        # batch columns per device tile
NB = BC // NT                   # 16 batch tiles per core
KT1 = (IN + 127) // 128         # 16 k-tiles for stage 1 (last is 80 rows)
HEADS_W = 512                   # [Wp|Wc|Wlat|Wum|pad8|Wh]

F32 = mybir.dt.float32
F32R = mybir.dt.float32r

LAST_RESULTS = None
_NC_CACHE = None


def round_fp32r(a: np.ndarray) -> np.ndarray:
    """Round f32 to fp32r (1s+8e+11m in the top 20 bits), nearest-even."""
    u = np.ascontiguousarray(a, dtype=np.float32).view(np.uint32)
    lsb = (u >> 12) & 1
    u2 = (u + 0x7FF + lsb) & np.uint32(0xFFFFF000)
    return u2.view(np.float32)


def build_nc():
    nc = bacc.Bacc(None)

    # x arrives pre-permuted: [batch-tile b][partition p][k-tile k][col j]
    # so each tile's load is one DMA with a 32 KiB contiguous run per
    # partition (descriptor-efficient).  IN is zero-padded to KT1*128.
    xt_d = nc.dram_tensor("xt", [NB, 128, KT1, NT], F32R, kind="ExternalInput")
    w1_d = nc.dram_tensor("w1", [H1 // 128, 128, KT1, 128], F32R, kind="ExternalInput")
    w2_d = nc.dram_tensor("w2", [H1, H2], F32R, kind="ExternalInput")
    wh_d = nc.dram_tensor("wh", [H2, HEADS_W], F32R, kind="ExternalInput")
    cw2_d = nc.dram_tensor("cw2", [H2, NHOT * CH], F32R, kind="ExternalInput")
    esel_d = nc.dram_tensor("esel", [P, NHOT * CH], F32R, kind="ExternalInput")
    ones_d = nc.dram_tensor("ones48", [1, NHOT * CH], F32R, kind="ExternalInput")
    bias_d = nc.dram_tensor("biases", [128, 11], F32, kind="ExternalInput")

    ident_d = nc.dram_tensor("ident", [128, 128], F32R, kind="ExternalInput")

    pl_o = nc.dram_tensor("plT", [P, BC], F32, kind="ExternalOutput")
    cl_o = nc.dram_tensor("clT", [C, BC], F32, kind="ExternalOutput")
    lat_o = nc.dram_tensor("latT", [16, BC], F32, kind="ExternalOutput")
    um_o = nc.dram_tensor("umT", [2, BC], F32, kind="ExternalOutput")
    z_o = nc.dram_tensor("zT", [H2, BC], F32, kind="ExternalOutput")
    rm_o = nc.dram_tensor("resmT", [NHOT * CH, BC], F32, kind="ExternalOutput")
    mk_o = nc.dram_tensor("maskB", [NB, NT // 128, 128], F32, kind="ExternalOutput")

    AF = mybir.ActivationFunctionType
    AX = mybir.AxisListType
    OP = mybir.AluOpType

    with tile.TileContext(nc) as tc:
        with tc.tile_pool(name="wpool", bufs=1) as wpool, \
             tc.tile_pool(name="xpool", bufs=2) as xpool, \
             tc.tile_pool(name="apool", bufs=2) as apool, \
             tc.tile_pool(name="ps1", bufs=2, space="PSUM") as ps1p, \
             tc.tile_pool(name="ps2", bufs=1, space="PSUM") as ps2p, \
             tc.tile_pool(name="ps3", bufs=2, space="PSUM") as ps3p, \
             tc.tile_pool(name="ps4", bufs=1, space="PSUM") as ps4p, \
             tc.tile_pool(name="psm", bufs=1, space="PSUM") as psmp, \
             tc.tile_pool(name="psmb", bufs=1, space="PSUM") as psmb:

            # ---- stage-1 weights + biases first; the rest are deferred
            # until after tile 0's stage-1 is emitted, so the first matmul
            # only waits on ~5 MB of DMA instead of the full 13 MB.
            bias_sb = wpool.tile([128, 11], F32, tag="biases")
            nc.sync.dma_start(out=bias_sb[:], in_=bias_d[:])
            w1_sb = []
            for m in range(H1 // 128):
                t = wpool.tile([128, KT1, 128], F32R, tag=f"w1_{m}")
                nc.sync.dma_start(out=t[:], in_=w1_d[m])
                w1_sb.append(t)

            w2_sb, wh_sb, cw2_sb = [], [], []
            esel_sb, ones_sb, ident_sb = None, None, None

            def emit_tail_weights():
                nonlocal esel_sb, ones_sb, ident_sb
                for k in range(H1 // 128):
                    t = wpool.tile([128, H2], F32R, tag=f"w2_{k}")
                    nc.sync.dma_start(out=t[:], in_=w2_d[k * 128:(k + 1) * 128])
                    w2_sb.append(t)
                for k in range(H2 // 128):
                    t = wpool.tile([128, HEADS_W], F32R, tag=f"wh_{k}")
                    nc.sync.dma_start(out=t[:], in_=wh_d[k * 128:(k + 1) * 128])
                    wh_sb.append(t)
                for k in range(H2 // 128):
                    t = wpool.tile([128, NHOT * CH], F32R, tag=f"cw2_{k}")
                    nc.sync.dma_start(out=t[:], in_=cw2_d[k * 128:(k + 1) * 128])
                    cw2_sb.append(t)
                esel_sb = wpool.tile([P, NHOT * CH], F32R, tag="esel")
                nc.sync.dma_start(out=esel_sb[:], in_=esel_d[:])
                ones_sb = wpool.tile([1, NHOT * CH], F32R, tag="ones48")
                nc.sync.dma_start(out=ones_sb[:], in_=ones_d[:])
                ident_sb = wpool.tile([128, 128], F32R, tag="ident")
                nc.sync.dma_start(out=ident_sb[:], in_=ident_d[:])

            # ---- batch tiles (software-pipelined emission) ----
            # PE program order per wall-step t:  s1(t) | s2(t-1) | s3+(t-2)
            # so the PE never waits on the ACT eviction latency at stage
            # boundaries — the next tile's stage-1 fills the gap.
            st = {}

            def emit_s1(b):
                xt_sb = xpool.tile([128, KT1, NT], F32R, tag="xt")
                nc.sync.dma_start(out=xt_sb[:], in_=xt_d[b])
                z1_sb = []
                for m in range(H1 // 128):
                    ps = ps1p.tile([128, NT], F32, tag="ps1")
                    for k in range(KT1):
                        nc.tensor.matmul(ps[:], w1_sb[m][:, k, :],
                                         xt_sb[:, k, :], start=(k == 0), stop=(k == KT1 - 1))
                    t = apool.tile([128, NT], F32R, tag=f"z1_{m}")
                    nc.vector.tensor_scalar(t[:], ps[:], bias_sb[:, m:m + 1], 0.0,
                                            OP.add, OP.max)
                    z1_sb.append(t)
                st[b] = {"z1": z1_sb}

            def emit_s2(b):
                z1_sb = st[b]["z1"]
                z_sb = []
                for m in range(H2 // 128):
                    ps = ps2p.tile([128, NT], F32, tag="ps2")
                    for k in range(H1 // 128):
                        nc.tensor.matmul(ps[:], w2_sb[k][:, m * 128:(m + 1) * 128],
                                         z1_sb[k][:], start=(k == 0), stop=(k == H1 // 128 - 1))
                    t = apool.tile([128, NT], F32R, tag=f"z_{m}")
                    nc.vector.tensor_scalar(t[:], ps[:], bias_sb[:, 4 + m:5 + m], 0.0,
                                            OP.add, OP.max)
                    z_sb.append(t)
                st[b]["z"] = z_sb

            def emit_s3rest(b):
                bs = slice(b * NT, (b + 1) * NT)
                z_sb = st.pop(b)["z"]

                # stage 3: headsT = Wheads.T @ zT + bheads  [512, NT] as 4 tiles
                # rows: 0:30 pl | 30:230 cl | 230:246 lat | 246:248 um | 248:256 pad | 256:512 pre_h
                heads_sb = []
                for m in range(HEADS_W // 128):
                    ps = ps3p.tile([128, NT], F32, tag="ps3")
                    for k in range(H2 // 128):
                        nc.tensor.matmul(ps[:], wh_sb[k][:, m * 128:(m + 1) * 128],
                                         z_sb[k][:], start=(k == 0), stop=(k == H2 // 128 - 1))
                    if m == 0:
                        t = apool.tile([128, NT], F32R, tag="h0")
                        nc.scalar.activation(t[:], ps[:], AF.Identity, bias=bias_sb[:, 6:7])
                    elif m == 1:
                        t = apool.tile([128, NT], F32, tag="h1")
                        nc.scalar.activation(t[:], ps[:], AF.Identity, bias=bias_sb[:, 7:8])
                    else:
                        t = apool.tile([128, NT], F32R, tag=f"h{m}")
                        nc.scalar.activation(t[:], ps[:], AF.Relu, bias=bias_sb[:, 6 + m:7 + m])
                    heads_sb.append(t)

                # stage 4: res48T = cW2blk.T @ hT + cb2   [48, NT]
                ps = ps4p.tile([NHOT * CH, NT], F32, tag="ps4")
                for k in range(H2 // 128):
                    nc.tensor.matmul(ps[:], cw2_sb[k][:], heads_sb[2 + k][:],
                                     start=(k == 0), stop=(k == H2 // 128 - 1))
                res48 = apool.tile([NHOT * CH, NT], F32, tag="res48")
                nc.scalar.activation(res48[:], ps[:], AF.Identity,
                                     bias=bias_sb[:NHOT * CH, 10:11])

                # mask path: argmax over the 30 parent logits, per batch column.
                # pl48 row j = parent_logit[hotspot_ids[j//CH]] (via esel matmul);
                # the max itself is computed in transposed orientation (PE
                # transpose -> DVE free-dim reduce), then broadcast back with
                # K=1 matmuls.  No gpsimd.
                pl = heads_sb[0]
                pl48_ps = psmp.tile([NHOT * CH, NT], F32, tag="psm")
                nc.tensor.matmul(pl48_ps[:], esel_sb[:], pl[:P], start=True, stop=True)
                pl48 = apool.tile([NHOT * CH, NT], F32, tag="pl48")
                nc.scalar.activation(pl48[:], pl48_ps[:], AF.Copy)

                nchunk = NT // 128
                plT_ps = psmb.tile([128, nchunk * P], F32, tag="psb")
                for j in range(nchunk):
                    nc.tensor.transpose(plT_ps[:, j * P:(j + 1) * P],
                                        pl[:P, j * 128:(j + 1) * 128].bitcast(F32),
                                        ident_sb[:P, :P].bitcast(F32))
                plT_v = plT_ps[:].rearrange("p (j c) -> p j c", c=P)
                max30T = apool.tile([128, nchunk], F32, tag="max30T")
                nc.vector.tensor_reduce(max30T[:], plT_v, AX.X, OP.max)
                hot4T = apool.tile([128, nchunk], F32, tag="hot4T")
                nc.vector.tensor_reduce(hot4T[:], plT_v[:, :, 0:NHOT], AX.X, OP.max)
                activeT = apool.tile([128, nchunk], F32, tag="activeT")
                nc.vector.tensor_tensor(activeT[:], hot4T[:], max30T[:], OP.is_ge)

                m30row_ps = psmp.tile([1, NT], F32, tag="psm")
                for j in range(nchunk):
                    nc.tensor.transpose(m30row_ps[:, j * 128:(j + 1) * 128],
                                        max30T[:, j:j + 1], ident_sb[:, :].bitcast(F32))
                m30row_sb = apool.tile([1, NT], F32R, tag="m30")
                nc.scalar.activation(m30row_sb[:], m30row_ps[:], AF.Copy)

                max48_ps = psmp.tile([NHOT * CH, NT], F32, tag="psm")
                nc.tensor.matmul(max48_ps[:], ones_sb[:], m30row_sb[:], start=True, stop=True)

                mask48 = apool.tile([NHOT * CH, NT], F32, tag="mask48")
                nc.vector.tensor_tensor(mask48[:], pl48[:], max48_ps[:], OP.is_equal)
                resm = apool.tile([NHOT * CH, NT], F32, tag="resm")
                nc.vector.tensor_mul(resm[:], mask48[:], res48[:])

                # outputs
                nc.sync.dma_start(out=pl_o[:, bs], in_=heads_sb[0][0:P].bitcast(F32))
                nc.sync.dma_start(out=cl_o[0:98, bs], in_=heads_sb[0][P:128].bitcast(F32))
                nc.sync.dma_start(out=cl_o[98:C, bs], in_=heads_sb[1][0:102])
                nc.sync.dma_start(out=lat_o[:, bs], in_=heads_sb[1][102:118])
                nc.sync.dma_start(out=um_o[:, bs], in_=heads_sb[1][118:120])
                nc.sync.dma_start(out=z_o[0:128, bs], in_=z_sb[0][:].bitcast(F32))
                nc.sync.dma_start(out=z_o[128:H2, bs], in_=z_sb[1][:].bitcast(F32))
                nc.sync.dma_start(out=rm_o[:, bs], in_=resm[:])
                for j in range(NT // 128):
                    nc.sync.dma_start(out=mk_o[b, j, :], in_=activeT[:, j:j + 1])

            for t_ in range(NB + 2):
                if t_ < NB:
                    emit_s1(t_)
                if t_ == 0:
                    emit_tail_weights()
                if 0 <= t_ - 1 < NB:
                    emit_s2(t_ - 1)
                if 0 <= t_ - 2 < NB:
                    emit_s3rest(t_ - 2)

    nc.finalize()
    return nc


def _prep_shared(W1, b1, W2, b2, Wp, bp, Wc, bc, Wlat, blat, Wum, bum,
                 cW1, cb1, cW2, cb2, hotspot_ids, child_idx):
    """Fold correction MLP layer-1 into a single z-matmul; pack weights/biases."""
    f8 = np.float64
    W1, W2, Wp, Wc, Wlat, Wum = [np.asarray(a, np.float32) for a in (W1, W2, Wp, Wc, Wlat, Wum)]
    cW1_, cW2_ = np.asarray(cW1, f8), np.asarray(cW2, f8)
    child = np.asarray(child_idx).reshape(NHOT, CH)
    hots = np.asarray(hotspot_ids).reshape(NHOT)

    Wh = np.zeros((H2, NHOT * FH), f8)
    bh = np.zeros(NHOT * FH, f8)
    cw2blk = np.zeros((NHOT * FH, NHOT * CH), f8)
    cb2flat = np.zeros(NHOT * CH, f8)
    for n in range(NHOT):
        A_n = cW1_[n, :H2, :]
        B_n = cW1_[n, H2:H2 + P, :]
        C_n = cW1_[n, H2 + P:, :]
        Wh[:, n * FH:(n + 1) * FH] = (A_n + np.asarray(Wp, f8) @ B_n
                                      + np.asarray(Wc, f8)[:, child[n]] @ C_n)
        bh[n * FH:(n + 1) * FH] = (np.asarray(bp, f8) @ B_n
                                   + np.asarray(bc, f8)[child[n]] @ C_n
                                   + np.asarray(cb1, f8)[n])
        cw2blk[n * FH:(n + 1) * FH, n * CH:(n + 1) * CH] = cW2_[n]
        cb2flat[n * CH:(n + 1) * CH] = np.asarray(cb2, f8)[n]

    wheads = np.zeros((H2, HEADS_W), np.float32)
    wheads[:, 0:P] = Wp
    wheads[:, P:P + C] = Wc
    wheads[:, 230:246] = Wlat
    wheads[:, 246:248] = Wum
    wheads[:, 256:512] = Wh.astype(np.float32)
    bheads = np.zeros(HEADS_W, np.float32)
    bheads[0:P] = bp
    bheads[P:P + C] = bc
    bheads[230:246] = blat
    bheads[246:248] = bum
    bheads[256:512] = bh.astype(np.float32)

    assert np.array_equal(hots, np.arange(NHOT)), (
        "device mask path hardcodes hotspot_ids == arange(NHOT)")
    esel = np.zeros((P, NHOT * CH), np.float32)
    for j in range(NHOT * CH):
        esel[hots[j // CH], j] = 1.0

    biases = np.zeros((128, 11), np.float32)
    for m in range(4):
        biases[:, m] = np.asarray(b1, np.float32)[m * 128:(m + 1) * 128]
    for m in range(2):
        biases[:, 4 + m] = np.asarray(b2, np.float32)[m * 128:(m + 1) * 128]
    for m in range(4):
        biases[:, 6 + m] = bheads[m * 128:(m + 1) * 128]
    biases[:NHOT * CH, 10] = cb2flat.astype(np.float32)

    w1p = np.zeros((KT1 * 128, H1), np.float32)
    w1p[:IN] = round_fp32r(W1)
    # [k*128+p, m*128+c] -> [m, p, k, c]
    w1p = np.ascontiguousarray(
        w1p.reshape(KT1, 128, H1 // 128, 128).transpose(2, 1, 0, 3))

    return {
        "w1": w1p,
        "w2": round_fp32r(W2),
        "wh": round_fp32r(wheads),
        "cw2": round_fp32r(cw2blk.astype(np.float32)),
        "esel": esel,                      # 0/1: already fp32r-exact
        "ones48": np.ones((1, NHOT * CH), np.float32),
        "biases": biases,
        "ident": np.eye(128, dtype=np.float32),
    }, child


def kernel(x, W1, b1, W2, b2, Wp, bp, Wc, bc, Wlat, blat, Wum, bum,
           cW1, cb1, cW2, cb2, hotspot_ids, child_idx):
    global LAST_RESULTS, _NC_CACHE
    shared, child = _prep_shared(W1, b1, W2, b2, Wp, bp, Wc, bc, Wlat, blat,
                                 Wum, bum, cW1, cb1, cW2, cb2, hotspot_ids, child_idx)

    x = np.asarray(x, np.float32)
    in_maps = []
    for i in range(N_CORES):
        xs = round_fp32r(x[i * BC:(i + 1) * BC])          # [BC, IN]
        xp = np.zeros((BC, KT1 * 128), np.float32)
        xp[:, :IN] = xs
        # [b*NT+j, k*128+p] -> [b, p, k, j]
        xp = np.ascontiguousarray(
            xp.reshape(NB, NT, KT1, 128).transpose(0, 3, 2, 1))
        sh = dict(shared)
        sh["xt"] = xp
        in_maps.append(sh)

    if _NC_CACHE is None:
        _NC_CACHE = build_nc()
    res = run_bass_kernel_spmd(_NC_CACHE, in_maps, list(range(N_CORES)))
    LAST_RESULTS = res

    pl = np.concatenate([np.asarray(r["plT"]).T for r in res.results], axis=0)
    cl = np.concatenate([np.asarray(r["clT"]).T for r in res.results], axis=0)
    lat = np.concatenate([np.asarray(r["latT"]).T for r in res.results], axis=0)
    um = np.concatenate([np.asarray(r["umT"]).T for r in res.results], axis=0)
    z = np.concatenate([np.asarray(r["zT"]).T for r in res.results], axis=0)
    resm = np.concatenate([np.asarray(r["resmT"]).T for r in res.results], axis=0)
    mask = np.concatenate([np.asarray(r["maskB"]).reshape(BC) for r in res.results], axis=0)

    corrected = cl.copy()
    corrected[:, child.ravel()] += resm
    active = mask > 0.5
    return pl, corrected, lat, um, z, active
